# revision 2
# baseline (speedup 1.0000x reference)
"""AgentAttention kernel for 8 axon-tunneled TRN2 NeuronCores.

Strategy (per spec sharding_hint): data-parallel over batch. The full batch
B=64 is split into 8 shards of 8; each NeuronCore runs the whole
AgentAttention forward on its shard with all params replicated. Outputs are
gathered back into the full (64, 785, 768) array.

Self-contained: hardcodes all shapes; reads nothing from disk.
"""

import numpy as np
import jax
import jax.numpy as jnp

DIM = 768
NUM_HEADS = 12
AGENT_NUM = 49
WINDOW = 28
POOL = 7
B = 64
N = 1 + WINDOW * WINDOW  # 785
N_CORES = 8

PARAM_NAMES = (
    "w_qkv", "w_proj", "b_proj", "dwc_w", "dwc_b",
    "an_bias", "ah_bias", "aw_bias", "na_bias", "ha_bias", "wa_bias",
    "ac_bias", "ca_bias",
)


def _forward(x, w_qkv, w_proj, b_proj, dwc_w, dwc_b, an_bias, ah_bias, aw_bias,
             na_bias, ha_bias, wa_bias, ac_bias, ca_bias):
    """AgentAttention forward on one batch shard (b, N, C). Mirrors reference."""
    b, n, c = x.shape
    H, hd, A, hw = NUM_HEADS, c // NUM_HEADS, AGENT_NUM, WINDOW
    scale = hd ** -0.5

    qkv = x @ w_qkv.T                                # (b, n, 3c)
    q, k, v = jnp.split(qkv, 3, axis=-1)

    s = hw // POOL
    qi = q[:, 1:, :].reshape(b, POOL, s, POOL, s, c)
    agent = qi.mean(axis=(2, 4)).reshape(b, A, c)    # (b, A, c)

    def heads(t, L):
        return t.reshape(b, L, H, hd).transpose(0, 2, 1, 3)

    qh = heads(q, n)
    kh = heads(k, n)
    vh = heads(v, n)
    agenth = heads(agent, A)                         # (b, H, A, hd)

    # stage 1: agents attend to K/V
    pb1 = jax.image.resize(an_bias, (H, A, hw, hw), method='bilinear')
    pb1 = pb1.reshape(1, H, A, hw * hw)
    pb2 = (ah_bias + aw_bias).reshape(1, H, A, hw * hw)
    pos_bias = jnp.concatenate([ac_bias, pb1 + pb2], axis=-1)   # (1, H, A, n)
    agent_attn = jax.nn.softmax(
        jnp.einsum('bhad,bhnd->bhan', agenth * scale, kh) + pos_bias, axis=-1)
    agent_v = agent_attn @ vh                        # (b, H, A, hd)

    # stage 2: queries attend to agents
    ab1 = jax.image.resize(na_bias, (H, A, hw, hw), method='bilinear')
    ab1 = ab1.reshape(1, H, A, hw * hw).transpose(0, 1, 3, 2)
    ab2 = (ha_bias + wa_bias).reshape(1, H, hw * hw, A)
    agent_bias = jnp.concatenate([ca_bias, ab1 + ab2], axis=-2)  # (1, H, n, A)
    q_attn = jax.nn.softmax(
        jnp.einsum('bhnd,bhad->bhna', qh * scale, agenth) + agent_bias, axis=-1)
    out = (q_attn @ agent_v).transpose(0, 2, 1, 3).reshape(b, n, c)

    # depthwise conv on V image tokens, added back
    vi = vh[:, :, 1:, :].transpose(0, 2, 1, 3).reshape(b, hw, hw, c)
    dw = jax.lax.conv_general_dilated(
        vi, dwc_w, window_strides=(1, 1), padding='SAME',
        dimension_numbers=('NHWC', 'HWIO', 'NHWC'),
        feature_group_count=c) + dwc_b
    out = out.at[:, 1:, :].add(dw.reshape(b, hw * hw, c))

    return out @ w_proj.T + b_proj


_pmapped = None


def _axon_devices():
    try:
        devs = jax.devices("axon")
    except Exception:
        devs = [d for d in jax.devices() if d.platform != "cpu"] or jax.devices()
    return devs[:N_CORES]


def _get_pmapped():
    global _pmapped
    if _pmapped is None:
        _pmapped = jax.pmap(
            _forward,
            in_axes=(0,) + (None,) * len(PARAM_NAMES),
            devices=_axon_devices(),
        )
    return _pmapped


def kernel(**inputs) -> np.ndarray:
    x = np.asarray(inputs["x"], dtype=np.float32)
    params = [np.asarray(inputs[name], dtype=np.float32) for name in PARAM_NAMES]
    shards = x.reshape(N_CORES, B // N_CORES, N, DIM)
    fn = _get_pmapped()
    out = fn(shards, *params)
    return np.asarray(out).reshape(B, N, DIM)


if __name__ == "__main__":
    rng = np.random.default_rng(0)
    fake = {
        "x": rng.standard_normal((B, N, DIM), dtype=np.float32),
        "w_qkv": rng.standard_normal((3 * DIM, DIM), dtype=np.float32) * DIM ** -0.5,
        "w_proj": rng.standard_normal((DIM, DIM), dtype=np.float32) * DIM ** -0.5,
        "b_proj": np.zeros((DIM,), dtype=np.float32),
        "dwc_w": rng.standard_normal((3, 3, 1, DIM), dtype=np.float32) * 0.1,
        "dwc_b": np.zeros((DIM,), dtype=np.float32),
        "an_bias": rng.standard_normal((NUM_HEADS, AGENT_NUM, 7, 7), dtype=np.float32) * 0.02,
        "ah_bias": rng.standard_normal((1, NUM_HEADS, AGENT_NUM, WINDOW, 1), dtype=np.float32) * 0.02,
        "aw_bias": rng.standard_normal((1, NUM_HEADS, AGENT_NUM, 1, WINDOW), dtype=np.float32) * 0.02,
        "na_bias": rng.standard_normal((NUM_HEADS, AGENT_NUM, 7, 7), dtype=np.float32) * 0.02,
        "ha_bias": rng.standard_normal((1, NUM_HEADS, WINDOW, 1, AGENT_NUM), dtype=np.float32) * 0.02,
        "wa_bias": rng.standard_normal((1, NUM_HEADS, 1, WINDOW, AGENT_NUM), dtype=np.float32) * 0.02,
        "ac_bias": rng.standard_normal((1, NUM_HEADS, AGENT_NUM, 1), dtype=np.float32) * 0.02,
        "ca_bias": rng.standard_normal((1, NUM_HEADS, 1, AGENT_NUM), dtype=np.float32) * 0.02,
    }
    out = kernel(**fake)
    print("kernel out", out.shape, out.dtype, float(np.abs(out).mean()))
